# revision 42
# baseline (speedup 1.0000x reference)
"""BEiT-style transformer block (prenorm attn w/ rel-pos bias + layerscale,
prenorm MLP w/ layerscale) on 8 Trainium2 NeuronCores, data-parallel over batch
(8 batches/core, no collectives).

Layout: feature-major activations [C, tokens] on chip so every big GEMM's
contraction dim (features) sits on the partition axis; host does the (free)
input/output transposes.  Big GEMMs run in float32r (full PE rate at
free-dim >= 256, ~1.5e-4 rel precision); attention internals run in bf16.

Attention uses the S^T = K @ Q^T form, both batches of a head processed as
one 394-column block: softmax needs no max-subtraction (scores are bounded),
the host-gathered relative-position bias is accumulated into the scores PSUM
with an identity matmul, exp runs on ACT, the denominator comes from a ones
column appended to token-major V, and 1/denom is broadcast across partitions
with a K=1 matmul.  LayerNorm stats are computed with all-ones matmuls
(partition reduce + broadcast in one shot), rsqrt(var+eps) =
exp(-0.5*ln(var+eps)) keeps ACT inside one activation-table set, and the
next chunk's LN is software-pipelined under the current chunk's attention.

Two phases (attention weights resident, then MLP weights resident), each in
its own TileContext so the boundary drain resets semaphore fan-in; the
residual stream crosses phases through a DRAM scratch tensor.  Modeled
(InstructionCostModel timeline) at ~466 us/core; measured rel err vs the
fp32 jax reference: 2.0e-4.
"""

import os
import sys

import numpy as np

for _p in ("/opt/trn_rl_repo",):
    if _p not in sys.path and os.path.isdir(_p):
        sys.path.insert(0, _p)

import ml_dtypes

import concourse.bass as bass
import concourse.bacc as bacc
import concourse.tile as tile
from concourse import mybir
from concourse.masks import make_identity

F32 = mybir.dt.float32
F32R = mybir.dt.float32r
BF16 = mybir.dt.bfloat16

# The act-table-load chooser first-matches Exp -> exp_and_others and
# Ln -> natural_log, bouncing tables (~2.7us each) on every layernorm's
# rsqrt = exp(-0.5*ln(var)).  Steer both to natural_log_exp_and_others
# (which holds exp AND ln) by hiding them from the single-function sets.
# Set ids (list positions) are preserved - only membership is filtered.
_orig_get_tables = bacc.get_activation_tables


def _patched_get_tables(arch):
    tabs = dict(_orig_get_tables(arch))
    A = mybir.ActivationFunctionType
    out = {}
    for name, fns in tabs.items():
        fns = set(fns)
        if name != "natural_log_exp_and_others":
            fns.discard(A.Exp)
            fns.discard(A.Ln)
        out[name] = fns
    return out


bacc.get_activation_tables = _patched_get_tables

# Problem shape (hardcoded per contract)
B = 64
N = 197          # tokens (14*14 + CLS)
C = 768          # embed dim
H = 12           # heads
HD = 64          # head dim
MLP = 3072
NCORES = 8
BLOC = B // NCORES          # 8 batches per core
TLOC = BLOC * N             # 1576 tokens per core
CH = 2 * N                  # 394-token chunks (2 batches) -> fp32r full rate
NCHUNK = BLOC // 2          # 4 chunks
KC = C // 128               # 6 feature chunks of 128
QKV_M = 3 * C // 128        # 18 qkv output chunks
MLP_K = MLP // 128          # 24 mlp hidden chunks
LN_EPS = 1e-5
SCALE = HD ** -0.5

_CACHE = {}
LAST_RESULTS = None


def _emit_ln_stats(nc, pool, pspool, x_t, bufs=2, stat_tag="st"):
    """LN stats over features: returns (mb, rst) [128, CH] broadcast tiles."""
    ps_sum = pspool.tile([128, CH], F32, tag=stat_tag)
    ps_ssq = pspool.tile([128, CH], F32, tag=stat_tag)
    allones = pool.allones_ref
    for k in range(KC):
        # walrus requires fp32r matmul operands to be produced as fp32r
        x_r = pool.tile([128, CH], F32R, tag="ln_xr")
        nc.vector.tensor_copy(x_r, x_t[:, k, :])
        xsq = pool.tile([128, CH], F32R, tag="ln_xsq")
        nc.scalar.activation(xsq, x_t[:, k, :],
                             mybir.ActivationFunctionType.Square)
        nc.tensor.matmul(
            ps_sum, allones[:, :], x_r[:, :],
            start=(k == 0), stop=(k == KC - 1))
        nc.tensor.matmul(
            ps_ssq, allones[:, :], xsq[:, :],
            start=(k == 0), stop=(k == KC - 1))
    return _stats_finish(nc, pool, ps_sum, ps_ssq, bufs)


def _stats_finish(nc, pool, ps_sum, ps_ssq, bufs=2):
    # all 128 partitions of ps_sum/ps_ssq hold the same column sums
    mb = pool.tile([128, CH], F32, tag="ln_mb", bufs=bufs)
    nc.vector.tensor_scalar_mul(mb, ps_sum, 1.0 / C)
    rst = pool.tile([128, CH], F32, tag="ln_rst", bufs=bufs)
    nc.vector.tensor_scalar_mul(rst, ps_ssq, 1.0 / C)
    m2 = pool.tile([128, CH], F32, tag="ln_m2")
    nc.vector.tensor_mul(m2, mb, mb)
    nc.vector.tensor_sub(rst, rst, m2)                       # var
    nc.scalar.activation(rst, rst, mybir.ActivationFunctionType.Ln,
                         bias=pool.eps_ref[:, :], scale=1.0)  # ln(var+eps)
    nc.scalar.activation(rst, rst, mybir.ActivationFunctionType.Exp,
                         scale=-0.5)                         # rsqrt(var+eps)
    return mb, rst


def _emit_ln_norm(nc, pool, x_t, mb, rst, w_sb, b_sb, tag):
    """h = (x - mb) * rst * w + b ; final affine runs on ACT (per-partition
    scale/bias).  Returns per-k float32r tiles so consumers can start on
    slice k=0 before the whole chunk is normalized."""
    hs = []
    for k in range(KC):
        t = pool.tile([128, CH], F32, tag="ln_t")
        nc.vector.tensor_sub(t, x_t[:, k, :], mb)
        nc.vector.tensor_mul(t, t, rst)
        hk = pool.tile([128, CH], F32R, tag=f"{tag}{k}", name=f"{tag}{k}")
        nc.scalar.activation(
            hk, t, mybir.ActivationFunctionType.Identity,
            bias=b_sb[:, k:k + 1], scale=w_sb[:, k:k + 1])
        hs.append(hk)
    return hs


def build_nc():
    nc = bacc.Bacc("TRN2")

    # ---- DRAM I/O (per-core shapes) ----
    xT = nc.declare_dram_parameter("xT", [C, TLOC], F32, isOutput=False)
    qkvwT = nc.declare_dram_parameter("qkvwT", [C, 3 * C], F32R, isOutput=False)
    projwT = nc.declare_dram_parameter("projwT", [C, C], F32R, isOutput=False)
    fc1wT = nc.declare_dram_parameter("fc1wT", [C, MLP], F32R, isOutput=False)
    fc2wT = nc.declare_dram_parameter("fc2wT", [MLP, C], F32R, isOutput=False)
    expbT = nc.declare_dram_parameter("expbT", [H, N, 2 * N], BF16,
                                      isOutput=False)
    vecs = {}
    for name, dim in [("qkvb", 3 * C), ("projb", C), ("g1", C),
                      ("n1w", C), ("n1b", C), ("n2w", C), ("n2b", C),
                      ("fc1b", MLP), ("fc2b", C), ("g2", C)]:
        vecs[name] = nc.declare_dram_parameter(name, [dim], F32, isOutput=False)
    epsv = nc.declare_dram_parameter("epsv", [128], F32, isOutput=False)
    onesw = nc.declare_dram_parameter("onesw", [128, 128], F32R, isOutput=False)
    xoutT = nc.declare_dram_parameter("xoutT", [C, TLOC], F32, isOutput=True)
    ffoutT = nc.declare_dram_parameter("ffoutT", [C, TLOC], F32, isOutput=True)
    xres_d = nc.dram_tensor("xres", [C, TLOC], F32)

    xT_ap = xT[:, :].rearrange("(k p) n -> p k n", p=128)
    xoutT_ap = xoutT[:, :].rearrange("(k p) n -> p k n", p=128)
    ffoutT_ap = ffoutT[:, :].rearrange("(k p) n -> p k n", p=128)
    xres_ap = xres_d[:, :].rearrange("(k p) n -> p k n", p=128)

    def load_vecs(pool, names):
        out = {}
        for name in names:
            dim = vecs[name].shape[0]
            t = pool.tile([128, dim // 128], F32, tag=f"v_{name}",
                          name=f"v_{name}")
            nc.sync.dma_start(
                out=t, in_=vecs[name][:].rearrange("(k p) -> p k", p=128))
            out[name] = t
        return out

    # ================= PHASE 1: attention =================
    # (separate TileContext per phase: the context exit drains + resets all
    # semaphores, keeping per-instruction sem-wait fan-in under the HW cap)
    with tile.TileContext(nc) as tc:
        with tc.tile_pool(name="consts", bufs=1) as consts, \
             tc.tile_pool(name="w1", bufs=1) as wpool, \
             tc.tile_pool(name="work1", bufs=2) as work, \
             tc.tile_pool(name="ps1", bufs=2, space="PSUM") as ps1:

            ident = consts.tile([128, 128], BF16)
            make_identity(nc, ident)
            ones64 = consts.tile([1, HD], BF16)
            nc.vector.memset(ones64, 1.0)
            allones = consts.tile([128, 128], F32R)
            nc.sync.dma_start(out=allones, in_=onesw[:, :])
            eps_t = consts.tile([128, 1], F32)
            nc.sync.dma_start(out=eps_t,
                              in_=epsv[:].rearrange("(k p) -> p k", p=128))

            sb = load_vecs(consts, ["qkvb", "projb", "g1", "n1w", "n1b"])
            gb1 = consts.tile([128, KC], F32)
            nc.vector.tensor_mul(gb1, sb["projb"], sb["g1"])
            # pre-scale the q part of the qkv bias by 1/sqrt(HD): the q rows
            # of the qkv matmul are evicted with scale=SCALE, bias included
            nc.vector.tensor_scalar_mul(
                sb["qkvb"][:, 0:KC], sb["qkvb"][:, 0:KC], SCALE)

            qkvw_sb = wpool.tile([128, KC, 3 * C], F32R)
            projw_sb = wpool.tile([128, KC, C], F32R)
            qkvw_ap = qkvwT[:, :].rearrange("(k p) m -> p k m", p=128)
            projw_ap = projwT[:, :].rearrange("(k p) m -> p k m", p=128)
            QBLK = 4 * 128
            for b0 in range(0, 3 * C, QBLK):
                be = min(b0 + QBLK, 3 * C)
                for k in range(KC):
                    nc.sync.dma_start(out=qkvw_sb[:, k, b0:be],
                                      in_=qkvw_ap[:, k, b0:be])
            eb0 = consts.tile([128, H, CH], BF16)
            eb1 = consts.tile([N - 128, H, CH], BF16)
            eb_ap = expbT[:, :, :].rearrange("h k q -> k h q")
            for h in range(H):
                nc.sync.dma_start(out=eb0[:, h, :], in_=eb_ap[0:128, h, :])
                nc.sync.dma_start(out=eb1[:, h, :], in_=eb_ap[128:N, h, :])
            for k in range(KC):
                nc.sync.dma_start(out=projw_sb[:, k, :], in_=projw_ap[:, k, :])

            work.allones_ref = allones
            work.eps_ref = eps_t

            def load_x(ci):
                x_c = work.tile([128, KC, CH], F32, tag="x", name=f"x_{ci}")
                for k in range(KC):
                    nc.scalar.dma_start(
                        out=x_c[:, k, :],
                        in_=xT_ap[:, k, ci * CH:(ci + 1) * CH])
                return x_c

            # software pipeline: chunk ci+1's LN stats are issued before
            # chunk ci's attention so the in-order DVE/PE queues overlap them
            x_tiles = {0: load_x(0)}
            stats = {0: _emit_ln_stats(nc, work, ps1, x_tiles[0])}
            h1s = {0: _emit_ln_norm(nc, work, x_tiles[0], *stats.pop(0),
                                    sb["n1w"], sb["n1b"], "h1_")}

            for ci in range(NCHUNK):
                c0 = ci * CH
                x_c = x_tiles.pop(ci)
                h1 = h1s.pop(ci)

                # qkv = h1 @ qkv_w.T + qkv_b   (feature-major, bf16 out)
                qkv_sb = work.tile([128, QKV_M, CH], BF16, tag="qkv", bufs=1)
                for m in range(QKV_M):
                    ps = ps1.tile([128, CH], F32, tag="f1")
                    for k in range(KC):
                        nc.tensor.matmul(
                            ps,
                            qkvw_sb[:, k, m * 128:(m + 1) * 128],
                            h1[k][:, :],
                            start=(k == 0), stop=(k == KC - 1))
                    nc.scalar.activation(
                        qkv_sb[:, m, :], ps,
                        mybir.ActivationFunctionType.Identity,
                        bias=sb["qkvb"][:, m:m + 1],
                        scale=SCALE if m < KC else 1.0)

                if ci + 1 < NCHUNK:
                    x_tiles[ci + 1] = load_x(ci + 1)
                    stats[ci + 1] = _emit_ln_stats(
                        nc, work, ps1, x_tiles[ci + 1])


                attnT = [work.tile([128, CH], F32R, tag=f"attnT{k}",
                                   name=f"attnT{k}", bufs=1)
                         for k in range(KC)]

                # token-major V for both batches (ones column appended)
                vts = []
                for b2 in range(2):
                    col0 = b2 * N
                    vt0 = work.tile([128, H, HD + 2], BF16, tag=f"vt0{b2}", bufs=1)
                    vt1 = work.tile([N - 128, H, HD + 2], BF16,
                                    tag=f"vt1{b2}", bufs=1)
                    nc.vector.memset(vt0[:, :, HD:HD + 1], 1.0)
                    nc.vector.memset(vt1[:, :, HD:HD + 1], 1.0)
                    for vc in range(KC):
                        for kc, (koff, klen) in enumerate(
                                [(0, 128), (128, N - 128)]):
                            pst = ps1.tile([128, 128], BF16, tag="st")
                            nc.tensor.transpose(
                                pst[:klen, :],
                                qkv_sb[:, 2 * KC + vc,
                                       col0 + koff:col0 + koff + klen],
                                ident[:, :])
                            vt = vt0 if kc == 0 else vt1
                            nc.vector.tensor_copy(
                                out=vt[:klen, 2 * vc:2 * vc + 2, 0:HD],
                                in_=pst[:klen, :].rearrange(
                                    "p (a b) -> p a b", a=2))
                    vts.append((vt0, vt1))

                # both batches of a head processed together: every op below
                # covers 2*N=394 columns, halving per-op fixed overheads
                for h in range(H):
                    ro = HD * (h % 2)
                    es = []
                    for kc, (koff, klen) in enumerate(
                            [(0, 128), (128, N - 128)]):
                        ps_s = ps1.tile([128, CH], F32, tag="ss")
                        for b2 in range(2):
                            col0 = b2 * N
                            qT = qkv_sb[ro:ro + HD, h // 2,
                                        col0:col0 + N]
                            kT = qkv_sb[ro:ro + HD, KC + h // 2,
                                        col0 + koff:col0 + koff + klen]
                            # one accumulation group per bank; has_written
                            # bits make the first write to each column range
                            # an overwrite
                            nc.tensor.matmul(
                                ps_s[:klen, col0:col0 + N],
                                kT, qT, start=(b2 == 0), stop=False)
                        eb = eb0 if kc == 0 else eb1
                        nc.tensor.matmul(
                            ps_s[:klen, :],
                            ident[:klen, :klen], eb[:klen, h, :],
                            start=False, stop=True)
                        e = work.tile([128, CH], BF16, tag=f"es{kc}")
                        nc.scalar.activation(
                            e[:klen, :], ps_s[:klen, :],
                            mybir.ActivationFunctionType.Exp)
                        es.append(e)
                    # attn @ v (+ones row): psum rows 0..63 = out^T,
                    # row 64 = softmax denominator
                    ps_o = ps1.tile([HD + 1, CH], F32, tag="so")
                    for b2 in range(2):
                        col0 = b2 * N
                        vt0, vt1 = vts[b2]
                        nc.tensor.matmul(
                            ps_o[:, col0:col0 + N], vt0[:, h, 0:HD + 1],
                            es[0][:, col0:col0 + N],
                            start=(b2 == 0), stop=False)
                        nc.tensor.matmul(
                            ps_o[:, col0:col0 + N], vt1[:, h, 0:HD + 1],
                            es[1][:N - 128, col0:col0 + N],
                            start=False, stop=(b2 == 1))
                    srb = work.tile([1, CH], BF16, tag="srb")
                    with nc.allow_low_precision(
                            reason="softmax denominator in bf16"):
                        nc.vector.reciprocal(srb, ps_o[HD:HD + 1, :])
                    ps_b = ps1.tile([HD, CH], F32, tag="st")
                    nc.tensor.matmul(ps_b, ones64[0:1, :], srb,
                                     start=True, stop=True)
                    rb = work.tile([HD, CH], F32, tag="rb")
                    if h % 2 == 0:
                        nc.vector.tensor_copy(rb, ps_b)
                    else:
                        nc.scalar.activation(
                            rb, ps_b, mybir.ActivationFunctionType.Copy)
                    nc.vector.tensor_mul(
                        attnT[h // 2][ro:ro + HD, :],
                        ps_o[0:HD, :], rb)

                # next chunk's LN normalize runs on DVE/ACT while the PE
                # is busy with this chunk's proj matmuls
                if ci + 1 < NCHUNK:
                    h1s[ci + 1] = _emit_ln_norm(
                        nc, work, x_tiles[ci + 1], *stats.pop(ci + 1),
                        sb["n1w"], sb["n1b"], "h1_")

                # proj + layerscale + residual -> xres (DRAM); the two
                # in-flight psum groups are interleaved over k so the PE has
                # runnable work while the last attention head drains
                for m0 in range(0, KC, 4):
                    nm = min(4, KC - m0)
                    pss = [ps1.tile([128, CH], F32,
                                    tag="f1" if d < 2 else "ss",
                                    name=f"projps{m0 + d}")
                           for d in range(nm)]
                    for k in range(KC):
                        for d in range(nm):
                            nc.tensor.matmul(
                                pss[d],
                                projw_sb[:, k, (m0 + d) * 128:(m0 + d + 1) * 128],
                                attnT[k][:, :],
                                start=(k == 0), stop=(k == KC - 1))
                    for d in range(nm):
                        m = m0 + d
                        po = work.tile([128, CH], F32, tag="po")
                        nc.scalar.activation(
                            po, pss[d], mybir.ActivationFunctionType.Identity,
                            bias=gb1[:, m:m + 1], scale=sb["g1"][:, m:m + 1])
                        xr = work.tile([128, CH], F32, tag="xr")
                        nc.vector.tensor_add(xr, po, x_c[:, m, :])
                        nc.sync.dma_start(
                            out=xres_ap[:, m, c0:c0 + CH], in_=xr)

    # ================= PHASE 2: MLP =================
    with tile.TileContext(nc) as tc:
        with tc.tile_pool(name="consts2", bufs=1) as consts2, \
             tc.tile_pool(name="w2", bufs=1) as wpool2, \
             tc.tile_pool(name="work2", bufs=2) as work2, \
             tc.tile_pool(name="ps2", bufs=2, space="PSUM") as ps2, \
             tc.tile_pool(name="psacc", bufs=1, space="PSUM") as psacc:

            sb = load_vecs(consts2, ["n2w", "n2b", "fc1b", "fc2b", "g2"])
            gb2 = consts2.tile([128, KC], F32)
            nc.vector.tensor_mul(gb2, sb["fc2b"], sb["g2"])
            allones = consts2.tile([128, 128], F32R)
            nc.sync.dma_start(out=allones, in_=onesw[:, :])
            eps_t = consts2.tile([128, 1], F32)
            nc.sync.dma_start(out=eps_t,
                              in_=epsv[:].rearrange("(k p) -> p k", p=128))
            work2.allones_ref = allones
            work2.eps_ref = eps_t

            fc1w_sb = wpool2.tile([128, KC, MLP], F32R)
            fc1w_ap = fc1wT[:, :].rearrange("(k p) m -> p k m", p=128)
            fc2w_sb = wpool2.tile([128, MLP_K, C], F32R)
            fc2w_ap = fc2wT[:, :].rearrange("(k p) m -> p k m", p=128)
            # stream weights in kk-blocks so fc1(kk=0) can start ~2us in
            BLK = 4 * 128
            for b0 in range(0, MLP, BLK):
                for k in range(KC):
                    nc.sync.dma_start(out=fc1w_sb[:, k, b0:b0 + BLK],
                                      in_=fc1w_ap[:, k, b0:b0 + BLK])
                for kk in range(b0 // 128, b0 // 128 + 4):
                    nc.sync.dma_start(out=fc2w_sb[:, kk, :],
                                      in_=fc2w_ap[:, kk, :])

            for ci in range(NCHUNK):
                c0 = ci * CH
                xr_c = work2.tile([128, KC, CH], F32, tag="xr2", bufs=1)
                for k in range(KC):
                    nc.scalar.dma_start(out=xr_c[:, k, :],
                                        in_=xres_ap[:, k, c0:c0 + CH])
                mb2, rst2 = _emit_ln_stats(nc, work2, ps2, xr_c,
                                           bufs=1, stat_tag="f1")
                h2 = _emit_ln_norm(nc, work2, xr_c, mb2, rst2,
                                   sb["n2w"], sb["n2b"], "h2_")

                acc = [psacc.tile([128, CH], F32, tag=f"fc2_{m}",
                                  name=f"fc2acc_{m}")
                       for m in range(KC)]
                for kk in range(MLP_K):
                    psf = ps2.tile([128, CH], F32, tag="f1")
                    for k in range(KC):
                        nc.tensor.matmul(
                            psf,
                            fc1w_sb[:, k, kk * 128:(kk + 1) * 128],
                            h2[k][:, :],
                            start=(k == 0), stop=(k == KC - 1))
                    hid = work2.tile([128, CH], F32R, tag="hid", bufs=2)
                    nc.scalar.activation(
                        hid, psf, mybir.ActivationFunctionType.Gelu,
                        bias=sb["fc1b"][:, kk:kk + 1], scale=1.0)
                    for m in range(KC):
                        nc.tensor.matmul(
                            acc[m],
                            fc2w_sb[:, kk, m * 128:(m + 1) * 128],
                            hid[:, :],
                            start=(kk == 0), stop=(kk == MLP_K - 1))
                for m in range(KC):
                    ff = work2.tile([128, CH], F32, tag="ff")
                    nc.scalar.activation(
                        ff, acc[m], mybir.ActivationFunctionType.Identity,
                        bias=gb2[:, m:m + 1], scale=sb["g2"][:, m:m + 1])
                    nc.sync.dma_start(
                        out=ffoutT_ap[:, m, c0:c0 + CH], in_=ff)
                    xo = work2.tile([128, CH], F32, tag="xo")
                    nc.vector.tensor_add(xo, ff, xr_c[:, m, :])
                    nc.sync.dma_start(
                        out=xoutT_ap[:, m, c0:c0 + CH], in_=xo)
    nc.finalize()
    return nc


def _prep_host(x, rel_pos_index, qkv_w, q_bias, v_bias, rpb_table, proj_w,
               proj_b, n1_w, n1_b, n2_w, n2_b, fc1_w, fc1_b, fc2_w, fc2_b,
               gamma1, gamma2):
    """Host-side input prep: transposes and the (constant) rel-pos gather."""
    f = np.float32
    bias = np.asarray(rpb_table, f)[np.asarray(rel_pos_index)]   # [N,N,H] (q,k,h)
    biasT = bias.transpose(2, 1, 0)                              # [H, key, q]
    biasT = np.concatenate([biasT, biasT], axis=2)               # 2 batches
    biasT = np.ascontiguousarray(biasT).astype(ml_dtypes.bfloat16)
    shared = {
        "qkvwT": np.ascontiguousarray(np.asarray(qkv_w, f).T),
        "projwT": np.ascontiguousarray(np.asarray(proj_w, f).T),
        "fc1wT": np.ascontiguousarray(np.asarray(fc1_w, f).T),
        "fc2wT": np.ascontiguousarray(np.asarray(fc2_w, f).T),
        "expbT": biasT,
        "qkvb": np.concatenate([np.asarray(q_bias, f),
                                np.zeros(C, f),
                                np.asarray(v_bias, f)]),
        "projb": np.asarray(proj_b, f), "g1": np.asarray(gamma1, f),
        "n1w": np.asarray(n1_w, f), "n1b": np.asarray(n1_b, f),
        "n2w": np.asarray(n2_w, f), "n2b": np.asarray(n2_b, f),
        "fc1b": np.asarray(fc1_b, f), "fc2b": np.asarray(fc2_b, f),
        "g2": np.asarray(gamma2, f),
        "epsv": np.full(128, LN_EPS, f),
        "onesw": np.ones((128, 128), f),
    }
    xT_all = np.ascontiguousarray(
        np.asarray(x, f).transpose(2, 0, 1).reshape(C, B * N))
    in_maps = []
    for i in range(NCORES):
        m = dict(shared)
        m["xT"] = np.ascontiguousarray(xT_all[:, i * TLOC:(i + 1) * TLOC])
        in_maps.append(m)
    return in_maps


def _get_runner():
    """Build (once) a cached jitted SPMD executable over 8 cores."""
    if "runner" in _CACHE:
        return _CACHE["runner"]
    import jax
    import jax.numpy as jnp
    from jax.sharding import Mesh, PartitionSpec
    from jax.experimental.shard_map import shard_map
    from concourse import bass2jax, mybir as mb

    nc = build_nc()
    bass2jax.install_neuronx_cc_hook()

    in_names, out_names, out_avals = [], [], []
    for alloc in nc.m.functions[0].allocations:
        if not isinstance(mb.MemoryLocationSet, type) or not isinstance(
                alloc, mb.MemoryLocationSet):
            continue
        name = alloc.memorylocations[0].name
        pname = (nc.partition_id_tensor.name
                 if nc.partition_id_tensor else None)
        if alloc.kind == "ExternalInput":
            if name != pname:
                in_names.append(name)
        elif alloc.kind == "ExternalOutput":
            out_names.append(name)
            out_avals.append(jax.core.ShapedArray(
                tuple(alloc.tensor_shape), mb.dt.np(alloc.dtype)))
    n_params = len(in_names)
    zero_outs = [np.zeros(a.shape, a.dtype) for a in out_avals]
    all_names = in_names + out_names
    if nc.partition_id_tensor is not None:
        all_names = all_names + [nc.partition_id_tensor.name]

    all_names_full = None

    def _body(*args):
        operands = list(args)
        if nc.partition_id_tensor is not None:
            operands.append(bass2jax.partition_id_tensor())
        outs = bass2jax._bass_exec_p.bind(
            *operands,
            out_avals=tuple(out_avals),
            in_names=tuple(all_names),
            out_names=tuple(out_names),
            lowering_input_output_aliases=(),
            sim_require_finite=True,
            sim_require_nnan=True,
            nc=nc,
        )
        return tuple(outs)

    devices = jax.devices()[:NCORES]
    mesh = Mesh(np.asarray(devices), ("core",))
    specs = (PartitionSpec("core"),) * (n_params + len(out_names))
    out_specs = (PartitionSpec("core"),) * len(out_names)
    fn = jax.jit(shard_map(_body, mesh=mesh, in_specs=specs,
                           out_specs=out_specs, check_rep=False),
                 keep_unused=True)
    _CACHE["runner"] = (fn, in_names, out_names, out_avals, zero_outs, mesh)
    return _CACHE["runner"]


def _run(in_maps):
    import jax
    from jax.sharding import NamedSharding, PartitionSpec
    fn, in_names, out_names, out_avals, zero_outs, mesh = _get_runner()
    concat_in = [np.concatenate([np.asarray(m[nm]) for m in in_maps], axis=0)
                 for nm in in_names]
    concat_zero = [np.zeros((NCORES * z.shape[0], *z.shape[1:]), z.dtype)
                   for z in zero_outs]
    sh = NamedSharding(mesh, PartitionSpec("core"))
    args = [jax.device_put(a, sh) for a in concat_in + concat_zero]
    out = fn(*args)
    jax.block_until_ready(out)
    _CACHE["last_args"] = args
    return {nm: np.asarray(out[i]).reshape(NCORES, *out_avals[i].shape)
            for i, nm in enumerate(out_names)}


def bench(iters=20):
    """Re-execute the cached executable; returns per-iteration seconds."""
    import time
    import jax
    fn, *_ = _get_runner()
    args = _CACHE["last_args"]
    times = []
    for _ in range(iters):
        t0 = time.perf_counter()
        out = fn(*args)
        jax.block_until_ready(out)
        times.append(time.perf_counter() - t0)
    return times


def kernel(**inputs):
    in_maps = _prep_host(**inputs)
    outs = _run(in_maps)
    x_out = np.concatenate([outs["xoutT"][i].reshape(C, BLOC, N)
                            for i in range(NCORES)], axis=1)
    ff_out = np.concatenate([outs["ffoutT"][i].reshape(C, BLOC, N)
                             for i in range(NCORES)], axis=1)
    return (np.ascontiguousarray(x_out.transpose(1, 2, 0)),
            np.ascontiguousarray(ff_out.transpose(1, 2, 0)))


# revision 45
# speedup vs baseline: 1.0557x; 1.0557x over previous
"""BEiT-style transformer block (prenorm attn w/ rel-pos bias + layerscale,
prenorm MLP w/ layerscale) on 8 Trainium2 NeuronCores, data-parallel over batch
(8 batches/core, no collectives).

Layout: feature-major activations [C, tokens] on chip so every big GEMM's
contraction dim (features) sits on the partition axis; host does the (free)
input/output transposes.  Big GEMMs run in float32r (full PE rate at
free-dim >= 256, ~1.5e-4 rel precision); attention internals run in bf16.

Attention uses the S^T = K @ Q^T form, both batches of a head processed as
one 394-column block: softmax needs no max-subtraction (scores are bounded),
the host-gathered relative-position bias is accumulated into the scores PSUM
with an identity matmul, exp runs on ACT, the denominator comes from a ones
column appended to token-major V, and 1/denom is broadcast across partitions
with a K=1 matmul.  LayerNorm stats are computed with all-ones matmuls
(partition reduce + broadcast in one shot), rsqrt(var+eps) =
exp(-0.5*ln(var+eps)) keeps ACT inside one activation-table set, and the
next chunk's LN is software-pipelined under the current chunk's attention.

Two phases (attention weights resident, then MLP weights resident), each in
its own TileContext so the boundary drain resets semaphore fan-in; the
residual stream crosses phases through a DRAM scratch tensor.  Modeled
(InstructionCostModel timeline) at ~466 us/core; measured rel err vs the
fp32 jax reference: 2.0e-4.
"""

import os
import sys

import numpy as np

for _p in ("/opt/trn_rl_repo",):
    if _p not in sys.path and os.path.isdir(_p):
        sys.path.insert(0, _p)

import ml_dtypes

import concourse.bass as bass
import concourse.bacc as bacc
import concourse.tile as tile
from concourse import mybir
from concourse.masks import make_identity

F32 = mybir.dt.float32
F32R = mybir.dt.float32r
BF16 = mybir.dt.bfloat16

# The act-table-load chooser first-matches Exp -> exp_and_others and
# Ln -> natural_log, bouncing tables (~2.7us each) on every layernorm's
# rsqrt = exp(-0.5*ln(var)).  Steer both to natural_log_exp_and_others
# (which holds exp AND ln) by hiding them from the single-function sets.
# Set ids (list positions) are preserved - only membership is filtered.
_orig_get_tables = bacc.get_activation_tables


def _patched_get_tables(arch):
    tabs = dict(_orig_get_tables(arch))
    A = mybir.ActivationFunctionType
    out = {}
    for name, fns in tabs.items():
        fns = set(fns)
        if name != "natural_log_exp_and_others":
            fns.discard(A.Exp)
            fns.discard(A.Ln)
        out[name] = fns
    return out


bacc.get_activation_tables = _patched_get_tables

# Problem shape (hardcoded per contract)
B = 64
N = 197          # tokens (14*14 + CLS)
C = 768          # embed dim
H = 12           # heads
HD = 64          # head dim
MLP = 3072
NCORES = 8
BLOC = B // NCORES          # 8 batches per core
TLOC = BLOC * N             # 1576 tokens per core
CH = 2 * N                  # 394-token chunks (2 batches) -> fp32r full rate
NCHUNK = BLOC // 2          # 4 chunks
KC = C // 128               # 6 feature chunks of 128
QKV_M = 3 * C // 128        # 18 qkv output chunks
MLP_K = MLP // 128          # 24 mlp hidden chunks
LN_EPS = 1e-5
SCALE = HD ** -0.5

_CACHE = {}
LAST_RESULTS = None


def _emit_ln_stats(nc, pool, pspool, x_t, bufs=2, stat_tag="st"):
    """LN stats over features: returns (mb, rst) [128, CH] broadcast tiles."""
    ps_sum = pspool.tile([128, CH], F32, tag=stat_tag)
    ps_ssq = pspool.tile([128, CH], F32, tag=stat_tag)
    allones = pool.allones_ref
    for k in range(KC):
        # walrus requires fp32r matmul operands to be produced as fp32r
        x_r = pool.tile([128, CH], F32R, tag="ln_xr")
        nc.vector.tensor_copy(x_r, x_t[:, k, :])
        xsq = pool.tile([128, CH], F32R, tag="ln_xsq")
        nc.scalar.activation(xsq, x_t[:, k, :],
                             mybir.ActivationFunctionType.Square)
        nc.tensor.matmul(
            ps_sum, allones[:, :], x_r[:, :],
            start=(k == 0), stop=(k == KC - 1))
        nc.tensor.matmul(
            ps_ssq, allones[:, :], xsq[:, :],
            start=(k == 0), stop=(k == KC - 1))
    return _stats_finish(nc, pool, ps_sum, ps_ssq, bufs)


def _stats_finish(nc, pool, ps_sum, ps_ssq, bufs=2):
    # all 128 partitions of ps_sum/ps_ssq hold the same column sums
    mb = pool.tile([128, CH], F32, tag="ln_mb", bufs=bufs)
    nc.vector.tensor_scalar_mul(mb, ps_sum, 1.0 / C)
    rst = pool.tile([128, CH], F32, tag="ln_rst", bufs=bufs)
    nc.vector.tensor_scalar_mul(rst, ps_ssq, 1.0 / C)
    m2 = pool.tile([128, CH], F32, tag="ln_m2")
    nc.vector.tensor_mul(m2, mb, mb)
    nc.vector.tensor_sub(rst, rst, m2)                       # var
    nc.scalar.activation(rst, rst, mybir.ActivationFunctionType.Ln,
                         bias=pool.eps_ref[:, :], scale=1.0)  # ln(var+eps)
    nc.scalar.activation(rst, rst, mybir.ActivationFunctionType.Exp,
                         scale=-0.5)                         # rsqrt(var+eps)
    return mb, rst


def _emit_ln_norm(nc, pool, x_t, mb, rst, w_sb, b_sb, tag):
    """h = (x - mb) * rst * w + b ; final affine runs on ACT (per-partition
    scale/bias).  Returns per-k float32r tiles so consumers can start on
    slice k=0 before the whole chunk is normalized."""
    hs = []
    for k in range(KC):
        t = pool.tile([128, CH], F32, tag="ln_t")
        nc.vector.tensor_sub(t, x_t[:, k, :], mb)
        nc.vector.tensor_mul(t, t, rst)
        hk = pool.tile([128, CH], F32R, tag=f"{tag}{k}", name=f"{tag}{k}")
        nc.scalar.activation(
            hk, t, mybir.ActivationFunctionType.Identity,
            bias=b_sb[:, k:k + 1], scale=w_sb[:, k:k + 1])
        hs.append(hk)
    return hs


def build_nc():
    nc = bacc.Bacc("TRN2")

    # ---- DRAM I/O (per-core shapes) ----
    xT = nc.declare_dram_parameter("xT", [C, TLOC], F32, isOutput=False)
    qkvwT = nc.declare_dram_parameter("qkvwT", [C, 3 * C], F32R, isOutput=False)
    projwT = nc.declare_dram_parameter("projwT", [C, C], F32R, isOutput=False)
    fc1wT = nc.declare_dram_parameter("fc1wT", [C, MLP], F32R, isOutput=False)
    fc2wT = nc.declare_dram_parameter("fc2wT", [MLP, C], F32R, isOutput=False)
    expbT = nc.declare_dram_parameter("expbT", [H, N, 2 * N], BF16,
                                      isOutput=False)
    vecs = {}
    for name, dim in [("qkvb", 3 * C), ("projb", C), ("g1", C),
                      ("n1w", C), ("n1b", C), ("n2w", C), ("n2b", C),
                      ("fc1b", MLP), ("fc2b", C), ("g2", C)]:
        vecs[name] = nc.declare_dram_parameter(name, [dim], F32, isOutput=False)
    epsv = nc.declare_dram_parameter("epsv", [128], F32, isOutput=False)
    onesw = nc.declare_dram_parameter("onesw", [128, 128], F32R, isOutput=False)
    xoutT = nc.declare_dram_parameter("xoutT", [C, TLOC], F32, isOutput=True)
    ffoutT = nc.declare_dram_parameter("ffoutT", [C, TLOC], F32, isOutput=True)
    xres_d = nc.dram_tensor("xres", [C, TLOC], F32)

    xT_ap = xT[:, :].rearrange("(k p) n -> p k n", p=128)
    xoutT_ap = xoutT[:, :].rearrange("(k p) n -> p k n", p=128)
    ffoutT_ap = ffoutT[:, :].rearrange("(k p) n -> p k n", p=128)
    xres_ap = xres_d[:, :].rearrange("(k p) n -> p k n", p=128)

    def load_vecs(pool, names):
        out = {}
        for name in names:
            dim = vecs[name].shape[0]
            t = pool.tile([128, dim // 128], F32, tag=f"v_{name}",
                          name=f"v_{name}")
            nc.sync.dma_start(
                out=t, in_=vecs[name][:].rearrange("(k p) -> p k", p=128))
            out[name] = t
        return out

    # ================= PHASE 1: attention =================
    # (separate TileContext per phase: the context exit drains + resets all
    # semaphores, keeping per-instruction sem-wait fan-in under the HW cap)
    with tile.TileContext(nc) as tc:
        with tc.tile_pool(name="consts", bufs=1) as consts, \
             tc.tile_pool(name="w1", bufs=1) as wpool, \
             tc.tile_pool(name="work1", bufs=2) as work, \
             tc.tile_pool(name="ps1", bufs=2, space="PSUM") as ps1:

            ident = consts.tile([128, 128], BF16)
            make_identity(nc, ident)
            ones64 = consts.tile([1, HD], BF16)
            nc.vector.memset(ones64, 1.0)
            allones = consts.tile([128, 128], F32R)
            nc.sync.dma_start(out=allones, in_=onesw[:, :])
            eps_t = consts.tile([128, 1], F32)
            nc.sync.dma_start(out=eps_t,
                              in_=epsv[:].rearrange("(k p) -> p k", p=128))
            # dummy Ln: triggers the natural_log_exp_and_others table load
            # immediately (under the weight DMAs) instead of inside the
            # first layernorm's critical chain
            warm = consts.tile([128, 1], F32)
            nc.scalar.activation(warm, eps_t,
                                 mybir.ActivationFunctionType.Ln)

            sb = load_vecs(consts, ["qkvb", "projb", "g1", "n1w", "n1b"])
            gb1 = consts.tile([128, KC], F32)
            nc.vector.tensor_mul(gb1, sb["projb"], sb["g1"])
            # pre-scale the q part of the qkv bias by 1/sqrt(HD): the q rows
            # of the qkv matmul are evicted with scale=SCALE, bias included
            nc.vector.tensor_scalar_mul(
                sb["qkvb"][:, 0:KC], sb["qkvb"][:, 0:KC], SCALE)

            qkvw_sb = wpool.tile([128, KC, 3 * C], F32R)
            projw_sb = wpool.tile([128, KC, C], F32R)
            qkvw_ap = qkvwT[:, :].rearrange("(k p) m -> p k m", p=128)
            projw_ap = projwT[:, :].rearrange("(k p) m -> p k m", p=128)
            QBLK = 4 * 128
            for b0 in range(0, 3 * C, QBLK):
                be = min(b0 + QBLK, 3 * C)
                for k in range(KC):
                    nc.sync.dma_start(out=qkvw_sb[:, k, b0:be],
                                      in_=qkvw_ap[:, k, b0:be])
            eb0 = consts.tile([128, H, CH], BF16)
            eb1 = consts.tile([N - 128, H, CH], BF16)
            eb_ap = expbT[:, :, :].rearrange("h k q -> k h q")
            for h in range(H):
                nc.sync.dma_start(out=eb0[:, h, :], in_=eb_ap[0:128, h, :])
                nc.sync.dma_start(out=eb1[:, h, :], in_=eb_ap[128:N, h, :])
            for k in range(KC):
                nc.sync.dma_start(out=projw_sb[:, k, :], in_=projw_ap[:, k, :])

            work.allones_ref = allones
            work.eps_ref = eps_t

            def load_x(ci):
                x_c = work.tile([128, KC, CH], F32, tag="x", name=f"x_{ci}")
                for k in range(KC):
                    nc.scalar.dma_start(
                        out=x_c[:, k, :],
                        in_=xT_ap[:, k, ci * CH:(ci + 1) * CH])
                return x_c

            # software pipeline: chunk ci+1's LN stats are issued before
            # chunk ci's attention so the in-order DVE/PE queues overlap them
            x_tiles = {0: load_x(0)}
            stats = {0: _emit_ln_stats(nc, work, ps1, x_tiles[0])}
            h1s = {0: _emit_ln_norm(nc, work, x_tiles[0], *stats.pop(0),
                                    sb["n1w"], sb["n1b"], "h1_")}

            for ci in range(NCHUNK):
                c0 = ci * CH
                x_c = x_tiles.pop(ci)
                h1 = h1s.pop(ci)

                # qkv = h1 @ qkv_w.T + qkv_b   (feature-major, bf16 out)
                qkv_sb = work.tile([128, QKV_M, CH], BF16, tag="qkv", bufs=1)
                for m in range(QKV_M):
                    ps = ps1.tile([128, CH], F32, tag="f1")
                    for k in range(KC):
                        nc.tensor.matmul(
                            ps,
                            qkvw_sb[:, k, m * 128:(m + 1) * 128],
                            h1[k][:, :],
                            start=(k == 0), stop=(k == KC - 1))
                    nc.scalar.activation(
                        qkv_sb[:, m, :], ps,
                        mybir.ActivationFunctionType.Identity,
                        bias=sb["qkvb"][:, m:m + 1],
                        scale=SCALE if m < KC else 1.0)

                if ci + 1 < NCHUNK:
                    x_tiles[ci + 1] = load_x(ci + 1)
                    stats[ci + 1] = _emit_ln_stats(
                        nc, work, ps1, x_tiles[ci + 1])


                attnT = [work.tile([128, CH], F32R, tag=f"attnT{k}",
                                   name=f"attnT{k}", bufs=1)
                         for k in range(KC)]

                # token-major V for both batches (ones column appended)
                vts = []
                for b2 in range(2):
                    col0 = b2 * N
                    vt0 = work.tile([128, H, HD + 2], BF16, tag=f"vt0{b2}", bufs=1)
                    vt1 = work.tile([N - 128, H, HD + 2], BF16,
                                    tag=f"vt1{b2}", bufs=1)
                    nc.vector.memset(vt0[:, :, HD:HD + 1], 1.0)
                    nc.vector.memset(vt1[:, :, HD:HD + 1], 1.0)
                    for vc in range(0, KC, 2):
                        # two feature chunks transpose into one psum bank so
                        # a single DVE op evicts four 64-col head slots
                        for kc, (koff, klen) in enumerate(
                                [(0, 128), (128, N - 128)]):
                            pst = ps1.tile([128, 256], BF16, tag="st")
                            for d in range(2):
                                nc.tensor.transpose(
                                    pst[:klen, d * 128:(d + 1) * 128],
                                    qkv_sb[:, 2 * KC + vc + d,
                                           col0 + koff:col0 + koff + klen],
                                    ident[:, :])
                            vt = vt0 if kc == 0 else vt1
                            nc.vector.tensor_copy(
                                out=vt[:klen, 2 * vc:2 * vc + 4, 0:HD],
                                in_=pst[:klen, :].rearrange(
                                    "p (a b) -> p a b", a=4))
                    vts.append((vt0, vt1))

                # both batches of a head processed together: every op below
                # covers 2*N=394 columns, halving per-op fixed overheads
                for h in range(H):
                    ro = HD * (h % 2)
                    es = []
                    for kc, (koff, klen) in enumerate(
                            [(0, 128), (128, N - 128)]):
                        ps_s = ps1.tile([128, CH], F32, tag="ss")
                        for b2 in range(2):
                            col0 = b2 * N
                            qT = qkv_sb[ro:ro + HD, h // 2,
                                        col0:col0 + N]
                            kT = qkv_sb[ro:ro + HD, KC + h // 2,
                                        col0 + koff:col0 + koff + klen]
                            # one accumulation group per bank; has_written
                            # bits make the first write to each column range
                            # an overwrite
                            nc.tensor.matmul(
                                ps_s[:klen, col0:col0 + N],
                                kT, qT, start=(b2 == 0), stop=False)
                        eb = eb0 if kc == 0 else eb1
                        nc.tensor.matmul(
                            ps_s[:klen, :],
                            ident[:klen, :klen], eb[:klen, h, :],
                            start=False, stop=True)
                        e = work.tile([128, CH], BF16, tag=f"es{kc}")
                        nc.scalar.activation(
                            e[:klen, :], ps_s[:klen, :],
                            mybir.ActivationFunctionType.Exp)
                        es.append(e)
                    # attn @ v (+ones row): psum rows 0..63 = out^T,
                    # row 64 = softmax denominator
                    ps_o = ps1.tile([HD + 1, CH], F32, tag="so")
                    for b2 in range(2):
                        col0 = b2 * N
                        vt0, vt1 = vts[b2]
                        nc.tensor.matmul(
                            ps_o[:, col0:col0 + N], vt0[:, h, 0:HD + 1],
                            es[0][:, col0:col0 + N],
                            start=(b2 == 0), stop=False)
                        nc.tensor.matmul(
                            ps_o[:, col0:col0 + N], vt1[:, h, 0:HD + 1],
                            es[1][:N - 128, col0:col0 + N],
                            start=False, stop=(b2 == 1))
                    srb = work.tile([1, CH], BF16, tag="srb")
                    with nc.allow_low_precision(
                            reason="softmax denominator in bf16"):
                        nc.vector.reciprocal(srb, ps_o[HD:HD + 1, :])
                    ps_b = ps1.tile([HD, CH], F32, tag="st")
                    nc.tensor.matmul(ps_b, ones64[0:1, :], srb,
                                     start=True, stop=True)
                    rb = work.tile([HD, CH], F32, tag="rb")
                    if h % 2 == 0:
                        nc.vector.tensor_copy(rb, ps_b)
                    else:
                        nc.scalar.activation(
                            rb, ps_b, mybir.ActivationFunctionType.Copy)
                    nc.vector.tensor_mul(
                        attnT[h // 2][ro:ro + HD, :],
                        ps_o[0:HD, :], rb)

                # next chunk's LN normalize runs on DVE/ACT while the PE
                # is busy with this chunk's proj matmuls
                if ci + 1 < NCHUNK:
                    h1s[ci + 1] = _emit_ln_norm(
                        nc, work, x_tiles[ci + 1], *stats.pop(ci + 1),
                        sb["n1w"], sb["n1b"], "h1_")

                # proj + layerscale + residual -> xres (DRAM); the two
                # in-flight psum groups are interleaved over k so the PE has
                # runnable work while the last attention head drains
                for m0 in range(0, KC, 4):
                    nm = min(4, KC - m0)
                    pss = [ps1.tile([128, CH], F32,
                                    tag="f1" if d < 2 else "ss",
                                    name=f"projps{m0 + d}")
                           for d in range(nm)]
                    for k in range(KC):
                        for d in range(nm):
                            nc.tensor.matmul(
                                pss[d],
                                projw_sb[:, k, (m0 + d) * 128:(m0 + d + 1) * 128],
                                attnT[k][:, :],
                                start=(k == 0), stop=(k == KC - 1))
                    for d in range(nm):
                        m = m0 + d
                        po = work.tile([128, CH], F32, tag="po", bufs=3)
                        nc.scalar.activation(
                            po, pss[d], mybir.ActivationFunctionType.Identity,
                            bias=gb1[:, m:m + 1], scale=sb["g1"][:, m:m + 1])
                        xr = work.tile([128, CH], F32, tag="xr", bufs=4)
                        nc.vector.tensor_add(xr, po, x_c[:, m, :])
                        nc.sync.dma_start(
                            out=xres_ap[:, m, c0:c0 + CH], in_=xr)

    # ================= PHASE 2: MLP =================
    with tile.TileContext(nc) as tc:
        with tc.tile_pool(name="consts2", bufs=1) as consts2, \
             tc.tile_pool(name="w2", bufs=1) as wpool2, \
             tc.tile_pool(name="work2", bufs=2) as work2, \
             tc.tile_pool(name="ps2", bufs=2, space="PSUM") as ps2, \
             tc.tile_pool(name="psacc", bufs=1, space="PSUM") as psacc:

            sb = load_vecs(consts2, ["n2w", "n2b", "fc1b", "fc2b", "g2"])
            gb2 = consts2.tile([128, KC], F32)
            nc.vector.tensor_mul(gb2, sb["fc2b"], sb["g2"])
            allones = consts2.tile([128, 128], F32R)
            nc.sync.dma_start(out=allones, in_=onesw[:, :])
            eps_t = consts2.tile([128, 1], F32)
            nc.sync.dma_start(out=eps_t,
                              in_=epsv[:].rearrange("(k p) -> p k", p=128))
            warm2 = consts2.tile([128, 1], F32)
            nc.scalar.activation(warm2, eps_t,
                                 mybir.ActivationFunctionType.Ln)
            work2.allones_ref = allones
            work2.eps_ref = eps_t

            fc1w_sb = wpool2.tile([128, KC, MLP], F32R)
            fc1w_ap = fc1wT[:, :].rearrange("(k p) m -> p k m", p=128)
            fc2w_sb = wpool2.tile([128, MLP_K, C], F32R)
            fc2w_ap = fc2wT[:, :].rearrange("(k p) m -> p k m", p=128)
            # stream weights in kk-blocks so fc1(kk=0) can start ~2us in
            BLK = 4 * 128
            for b0 in range(0, MLP, BLK):
                for k in range(KC):
                    nc.sync.dma_start(out=fc1w_sb[:, k, b0:b0 + BLK],
                                      in_=fc1w_ap[:, k, b0:b0 + BLK])
                for kk in range(b0 // 128, b0 // 128 + 4):
                    nc.sync.dma_start(out=fc2w_sb[:, kk, :],
                                      in_=fc2w_ap[:, kk, :])

            for ci in range(NCHUNK):
                c0 = ci * CH
                xr_c = work2.tile([128, KC, CH], F32, tag="xr2", bufs=1)
                for k in range(KC):
                    nc.scalar.dma_start(out=xr_c[:, k, :],
                                        in_=xres_ap[:, k, c0:c0 + CH])
                mb2, rst2 = _emit_ln_stats(nc, work2, ps2, xr_c,
                                           bufs=1, stat_tag="f1")
                h2 = _emit_ln_norm(nc, work2, xr_c, mb2, rst2,
                                   sb["n2w"], sb["n2b"], "h2_")

                acc = [psacc.tile([128, CH], F32, tag=f"fc2_{m}",
                                  name=f"fc2acc_{m}")
                       for m in range(KC)]
                for kk in range(MLP_K):
                    psf = ps2.tile([128, CH], F32, tag="f1")
                    for k in range(KC):
                        nc.tensor.matmul(
                            psf,
                            fc1w_sb[:, k, kk * 128:(kk + 1) * 128],
                            h2[k][:, :],
                            start=(k == 0), stop=(k == KC - 1))
                    hid = work2.tile([128, CH], F32R, tag="hid", bufs=2)
                    nc.scalar.activation(
                        hid, psf, mybir.ActivationFunctionType.Gelu,
                        bias=sb["fc1b"][:, kk:kk + 1], scale=1.0)
                    for m in range(KC):
                        nc.tensor.matmul(
                            acc[m],
                            fc2w_sb[:, kk, m * 128:(m + 1) * 128],
                            hid[:, :],
                            start=(kk == 0), stop=(kk == MLP_K - 1))
                for m in range(KC):
                    ff = work2.tile([128, CH], F32, tag="ff", bufs=3)
                    nc.scalar.activation(
                        ff, acc[m], mybir.ActivationFunctionType.Identity,
                        bias=gb2[:, m:m + 1], scale=sb["g2"][:, m:m + 1])
                    nc.sync.dma_start(
                        out=ffoutT_ap[:, m, c0:c0 + CH], in_=ff)
                    xo = work2.tile([128, CH], F32, tag="xo", bufs=3)
                    nc.vector.tensor_add(xo, ff, xr_c[:, m, :])
                    nc.sync.dma_start(
                        out=xoutT_ap[:, m, c0:c0 + CH], in_=xo)
    nc.finalize()
    return nc


def _prep_host(x, rel_pos_index, qkv_w, q_bias, v_bias, rpb_table, proj_w,
               proj_b, n1_w, n1_b, n2_w, n2_b, fc1_w, fc1_b, fc2_w, fc2_b,
               gamma1, gamma2):
    """Host-side input prep: transposes and the (constant) rel-pos gather."""
    f = np.float32
    bias = np.asarray(rpb_table, f)[np.asarray(rel_pos_index)]   # [N,N,H] (q,k,h)
    biasT = bias.transpose(2, 1, 0)                              # [H, key, q]
    biasT = np.concatenate([biasT, biasT], axis=2)               # 2 batches
    biasT = np.ascontiguousarray(biasT).astype(ml_dtypes.bfloat16)
    shared = {
        "qkvwT": np.ascontiguousarray(np.asarray(qkv_w, f).T),
        "projwT": np.ascontiguousarray(np.asarray(proj_w, f).T),
        "fc1wT": np.ascontiguousarray(np.asarray(fc1_w, f).T),
        "fc2wT": np.ascontiguousarray(np.asarray(fc2_w, f).T),
        "expbT": biasT,
        "qkvb": np.concatenate([np.asarray(q_bias, f),
                                np.zeros(C, f),
                                np.asarray(v_bias, f)]),
        "projb": np.asarray(proj_b, f), "g1": np.asarray(gamma1, f),
        "n1w": np.asarray(n1_w, f), "n1b": np.asarray(n1_b, f),
        "n2w": np.asarray(n2_w, f), "n2b": np.asarray(n2_b, f),
        "fc1b": np.asarray(fc1_b, f), "fc2b": np.asarray(fc2_b, f),
        "g2": np.asarray(gamma2, f),
        "epsv": np.full(128, LN_EPS, f),
        "onesw": np.ones((128, 128), f),
    }
    xT_all = np.ascontiguousarray(
        np.asarray(x, f).transpose(2, 0, 1).reshape(C, B * N))
    in_maps = []
    for i in range(NCORES):
        m = dict(shared)
        m["xT"] = np.ascontiguousarray(xT_all[:, i * TLOC:(i + 1) * TLOC])
        in_maps.append(m)
    return in_maps


def _get_runner():
    """Build (once) a cached jitted SPMD executable over 8 cores."""
    if "runner" in _CACHE:
        return _CACHE["runner"]
    import jax
    import jax.numpy as jnp
    from jax.sharding import Mesh, PartitionSpec
    from jax.experimental.shard_map import shard_map
    from concourse import bass2jax, mybir as mb

    nc = build_nc()
    bass2jax.install_neuronx_cc_hook()

    in_names, out_names, out_avals = [], [], []
    for alloc in nc.m.functions[0].allocations:
        if not isinstance(mb.MemoryLocationSet, type) or not isinstance(
                alloc, mb.MemoryLocationSet):
            continue
        name = alloc.memorylocations[0].name
        pname = (nc.partition_id_tensor.name
                 if nc.partition_id_tensor else None)
        if alloc.kind == "ExternalInput":
            if name != pname:
                in_names.append(name)
        elif alloc.kind == "ExternalOutput":
            out_names.append(name)
            out_avals.append(jax.core.ShapedArray(
                tuple(alloc.tensor_shape), mb.dt.np(alloc.dtype)))
    n_params = len(in_names)
    zero_outs = [np.zeros(a.shape, a.dtype) for a in out_avals]
    all_names = in_names + out_names
    if nc.partition_id_tensor is not None:
        all_names = all_names + [nc.partition_id_tensor.name]

    all_names_full = None

    def _body(*args):
        operands = list(args)
        if nc.partition_id_tensor is not None:
            operands.append(bass2jax.partition_id_tensor())
        outs = bass2jax._bass_exec_p.bind(
            *operands,
            out_avals=tuple(out_avals),
            in_names=tuple(all_names),
            out_names=tuple(out_names),
            lowering_input_output_aliases=(),
            sim_require_finite=True,
            sim_require_nnan=True,
            nc=nc,
        )
        return tuple(outs)

    devices = jax.devices()[:NCORES]
    mesh = Mesh(np.asarray(devices), ("core",))
    specs = (PartitionSpec("core"),) * (n_params + len(out_names))
    out_specs = (PartitionSpec("core"),) * len(out_names)
    fn = jax.jit(shard_map(_body, mesh=mesh, in_specs=specs,
                           out_specs=out_specs, check_rep=False),
                 keep_unused=True)
    _CACHE["runner"] = (fn, in_names, out_names, out_avals, zero_outs, mesh)
    return _CACHE["runner"]


def _run(in_maps):
    import jax
    from jax.sharding import NamedSharding, PartitionSpec
    fn, in_names, out_names, out_avals, zero_outs, mesh = _get_runner()
    concat_in = [np.concatenate([np.asarray(m[nm]) for m in in_maps], axis=0)
                 for nm in in_names]
    concat_zero = [np.zeros((NCORES * z.shape[0], *z.shape[1:]), z.dtype)
                   for z in zero_outs]
    sh = NamedSharding(mesh, PartitionSpec("core"))
    args = [jax.device_put(a, sh) for a in concat_in + concat_zero]
    out = fn(*args)
    jax.block_until_ready(out)
    _CACHE["last_args"] = args
    return {nm: np.asarray(out[i]).reshape(NCORES, *out_avals[i].shape)
            for i, nm in enumerate(out_names)}


def bench(iters=20):
    """Re-execute the cached executable; returns per-iteration seconds."""
    import time
    import jax
    fn, *_ = _get_runner()
    args = _CACHE["last_args"]
    times = []
    for _ in range(iters):
        t0 = time.perf_counter()
        out = fn(*args)
        jax.block_until_ready(out)
        times.append(time.perf_counter() - t0)
    return times


def kernel(**inputs):
    in_maps = _prep_host(**inputs)
    outs = _run(in_maps)
    x_out = np.concatenate([outs["xoutT"][i].reshape(C, BLOC, N)
                            for i in range(NCORES)], axis=1)
    ff_out = np.concatenate([outs["ffoutT"][i].reshape(C, BLOC, N)
                             for i in range(NCORES)], axis=1)
    return (np.ascontiguousarray(x_out.transpose(1, 2, 0)),
            np.ascontiguousarray(ff_out.transpose(1, 2, 0)))
